# revision 1
# baseline (speedup 1.0000x reference)
"""Bass/Trainium2 kernel for nn_BiGRIL (gnn_message_passing).

Key algebraic structure (valid because the reference's hidden state h is
identically zero and C == 1):
  x1   = where(mask, x, b_fs)
  z    = W0*x1 + W1*m + b_in            (rank-2 in channels!)
  zg   = A^T z  ->  W0*xg + W1*mg + b_in*cg   with xg = A^T x1, mg = A^T m,
                                              cg = A^T 1
  o    = PReLU(M1 z + M2 zg + b_fold)   (K=6 matmul over 6 data streams)
  xs2  = w_ro . o + b_ro
  y    = relu(W_o1 xs2 + b_o1)          (rank-1 K=64 matmul, relu fused)
  out  = W_o2 . y + b_o2                (K=64 contraction)

Sharding: pure data-parallel over batch (B=8 -> 8 cores), weights + adj
replicated.  No collectives.
"""

import numpy as np
import sys

sys.path.insert(0, "/opt/trn_rl_repo")

B, C, N, T = 8, 1, 1024, 64
H = 64
NT = N * T          # 65536 per-core output elements
CHUNK = 512         # psum-bank-sized column chunk
NCHUNK = NT // CHUNK
BLK = 4096          # movA tile columns (64 nodes x 64 steps)
NBLK = NT // BLK    # 16 blocks per core

_CACHE = {}


def _fold_weights(W_fs, b_fs, W_in, b_in, W_gc, b_gc, W_lo, b_lo, prelu_a,
                  W_ro, b_ro, W_o1, b_o1, W_o2, b_o2, adj):
    """Host-side weight folding in float64 for accuracy."""
    f8 = np.float64
    W_in, b_in = W_in.astype(f8), b_in.astype(f8)
    W_gc, b_gc = W_gc.astype(f8), b_gc.astype(f8)
    W_lo, b_lo = W_lo.astype(f8), b_lo.astype(f8)
    W_ro, b_ro = W_ro.astype(f8), b_ro.astype(f8)
    W_o1, b_o1 = W_o1.astype(f8), b_o1.astype(f8)
    W_o2, b_o2 = W_o2.astype(f8), b_o2.astype(f8)

    W0 = W_in[:, 0]           # x1 channel  [64]
    W1 = W_in[:, 1]           # mask channel [64]
    Wlo1 = W_lo[:, :H]
    M1 = Wlo1 @ W_gc[:, :H]
    M2 = Wlo1 @ W_gc[:, H:]
    b_fold = Wlo1 @ b_gc + b_lo

    PA = np.stack([
        M1 @ W0,
        M1 @ W1,
        M2 @ W0,
        M2 @ W1,
        M2 @ b_in,
        M1 @ b_in + b_fold,
    ])                                     # [6, 64]  lhsT for pass A

    w_ro1 = W_ro[0, :H]                    # [64]
    PB = np.outer(w_ro1, W_o1[:, 0])       # [64(h), 64(f)] lhsT for pass B
    bias_f = W_o1[:, 0] * b_ro[0] + b_o1   # [64]

    den = float(np.sum(W_o2[0] ** 2))
    if abs(den) < 1e-12:
        k = np.zeros(H)
        extra_const = float(b_o2[0])       # would need separate handling
    else:
        k = float(b_o2[0]) * W_o2[0] / den
        extra_const = 0.0
    assert extra_const == 0.0

    cg = adj.astype(f8).sum(axis=0)        # [N] column sums of adj
    cgrep = np.repeat(cg, T)               # [(m,t)] layout m*T + t

    a = float(prelu_a)
    assert 0.0 < a < 1.0
    # All stationaries are K=128 (zero-padded): the PE activity monitor
    # only un-throttles the clock for full-K matmuls.
    # moving layout (ma2): rows 0:6 = streams, 6:64 = zeros,
    #                      64:128 = t1 = (1-a)*relu(v)
    pa128 = np.zeros((128, H))
    pa128[0:6, :] = PA                      # phase A: v = PA^T streams
    pb128 = np.zeros((128, 128))
    pb128[0:6, 0:H] = a * (PA @ PB)         # linear PReLU branch
    pb128[64:128, 0:H] = PB                 # + PB^T t1
    # cols 64:128 stay zero -> psum_v rows 64:128 written as exact zeros
    pc128 = np.zeros((128, 32))
    pc128[0:H, 0] = W_o2[0]
    bk128 = np.zeros((128, 1))
    bk128[:H, 0] = bias_f + k
    kk128 = np.zeros((128, 1))
    kk128[:H, 0] = k

    fp = np.float32
    h16 = np.float16
    return dict(
        pa=pa128.astype(h16),
        pb=pb128.astype(h16),
        pc=pc128.astype(h16),
        bk=bk128.astype(fp),
        kk=kk128.astype(fp),
        bfs=np.full((128, 1), b_fs[0], fp),
        zr=np.zeros((122, BLK), h16),
        sc=np.full((128, 1), 1.0 - a, fp),
        cgrep=cgrep.astype(h16),
        ones_row=np.ones(BLK, h16),
        prelu_a=a,
    )


def _build_program():
    import concourse.bass as bass
    import concourse.bacc as bacc
    import concourse.mybir as mybir
    import concourse.tile as tile

    dt = mybir.dt
    f32 = dt.float32
    h16 = dt.float16
    AF = mybir.ActivationFunctionType
    ALU = mybir.AluOpType

    nc = bacc.Bacc("TRN2", target_bir_lowering=False, debug=False,
                   num_devices=B)

    xb = nc.dram_tensor("xb", [N, T], h16, kind="ExternalInput")
    mb = nc.dram_tensor("mb", [N, T], h16, kind="ExternalInput")
    adj = nc.dram_tensor("adj", [N, N], h16, kind="ExternalInput")
    cgrep = nc.dram_tensor("cgrep", [NT], h16, kind="ExternalInput")
    onesr = nc.dram_tensor("ones_row", [BLK], h16, kind="ExternalInput")
    zr_d = nc.dram_tensor("zr", [122, BLK], h16, kind="ExternalInput")
    pa_d = nc.dram_tensor("pa", [128, H], h16, kind="ExternalInput")
    pb_d = nc.dram_tensor("pb", [128, 128], h16, kind="ExternalInput")
    pc_d = nc.dram_tensor("pc", [128, 32], h16, kind="ExternalInput")
    bk_d = nc.dram_tensor("bk", [128, 1], f32, kind="ExternalInput")
    kk_d = nc.dram_tensor("kk", [128, 1], f32, kind="ExternalInput")
    bfs_d = nc.dram_tensor("bfs", [128, 1], f32, kind="ExternalInput")
    sc_d = nc.dram_tensor("sc", [128, 1], f32, kind="ExternalInput")
    out_d = nc.dram_tensor("out", [NT], f32, kind="ExternalOutput")

    MOVA_BUFS = 4
    RR_BUFS = 8
    from contextlib import ExitStack
    with tile.TileContext(nc) as tc, ExitStack() as ctx:
        const = ctx.enter_context(tc.tile_pool(name="const", bufs=1))
        adjp = ctx.enter_context(tc.tile_pool(name="adjp", bufs=1))
        gmovp = ctx.enter_context(tc.tile_pool(name="gmovp", bufs=1))
        movap = ctx.enter_context(tc.tile_pool(name="movap", bufs=1))
        rrp = ctx.enter_context(tc.tile_pool(name="rrp", bufs=RR_BUFS))
        posbp = ctx.enter_context(tc.tile_pool(name="posbp", bufs=3))
        psp = ctx.enter_context(tc.tile_pool(name="psp", bufs=6, space="PSUM"))
        pop = ctx.enter_context(tc.tile_pool(name="pop", bufs=2, space="PSUM"))

        pa_t = const.tile([128, H], h16)
        pb_t = const.tile([128, 128], h16)
        pc_t = const.tile([128, 32], h16)
        bk_t = const.tile([128, 1], f32)
        kk_t = const.tile([128, 1], f32)
        bfs_t = const.tile([128, 1], f32)
        sc_t = const.tile([128, 1], f32)
        nc.sync.dma_start(out=pa_t[:], in_=pa_d[:])
        nc.sync.dma_start(out=pb_t[:], in_=pb_d[:])
        nc.sync.dma_start(out=pc_t[:], in_=pc_d[:])
        nc.sync.dma_start(out=bk_t[:], in_=bk_d[:])
        nc.sync.dma_start(out=kk_t[:], in_=kk_d[:])
        nc.sync.dma_start(out=bfs_t[:], in_=bfs_d[:])
        nc.sync.dma_start(out=sc_t[:], in_=sc_d[:])

        # ---- x1/m streams (fp16) + adj tiles --------------------------
        gmov = []
        for nt in range(8):
            g = gmovp.tile([128, 128], h16, tag=f"gmov{nt}", name=f"gmov{nt}")
            nc.sync.dma_start(out=g[:, 0:64], in_=xb[nt * 128:(nt + 1) * 128, :])
            nc.sync.dma_start(out=g[:, 64:128], in_=mb[nt * 128:(nt + 1) * 128, :])
            nc.vector.scalar_tensor_tensor(
                out=g[:, 0:64], in0=g[:, 0:64], scalar=bfs_t[:, 0:1],
                in1=g[:, 64:128], op0=ALU.subtract, op1=ALU.mult)
            nc.vector.tensor_scalar_add(g[:, 0:64], g[:, 0:64], bfs_t[:, 0:1])
            gmov.append(g)
        adjt = []
        for nt in range(8):
            at = adjp.tile([128, 1024], h16, tag=f"adjt{nt}", name=f"adjt{nt}")
            nc.sync.dma_start(out=at[:], in_=adj[nt * 128:(nt + 1) * 128, :])
            adjt.append(at)

        # ---- software-pipelined G + tail over all 128 chunks ----------
        # ma2 layout: rows 0:6 streams, rows 6:64 zeros, rows 64:128 t1.
        # All matmuls K=128 (zero-padded stationaries) so the PE clock
        # monitor sees full-array activity and un-throttles to 2.4 GHz.
        # ma tiles are persistent (4, cycled per block): their zero rows
        # are initialized once; t1 rows are rewritten by ACT every chunk.
        ma4 = []
        for i in range(MOVA_BUFS):
            mai = movap.tile([128, BLK], h16, tag=f"mova{i}", name=f"mova{i}")
            nc.gpsimd.memset(mai[:, :], 0.0)
            nc.sync.dma_start(out=mai[5:6, :], in_=onesr[:])
            ma4.append(mai)
        gx = [None] * 8
        ma_t = [None] * NBLK
        pss = {}
        rrs = {}
        po_ps = {}

        def emit_g(mt):
            psg = psp.tile([128, 512], f32, tag="ps", name=f"psg{mt}")
            for nt in range(8):
                nc.tensor.matmul(
                    psg[:, 0:128],
                    adjt[nt][:, mt * 128:(mt + 1) * 128],
                    gmov[nt][:],
                    start=(nt == 0), stop=(nt == 7))
            g = gmovp.tile([128, 128], h16, tag=f"gxm{mt}", name=f"gxm{mt}")
            nc.vector.tensor_copy(g[:], psg[:, 0:128])
            gx[mt] = g

        def emit_ma(blk):
            mt, half = blk // 2, blk % 2
            p0 = half * 64
            ma = ma4[blk % MOVA_BUFS]
            nc.sync.dma_start(out=ma[0:1, :], in_=gmov[mt][p0:p0 + 64, 0:64])
            nc.sync.dma_start(out=ma[1:2, :], in_=gmov[mt][p0:p0 + 64, 64:128])
            nc.gpsimd.dma_start(out=ma[2:3, :], in_=gx[mt][p0:p0 + 64, 0:64])
            nc.gpsimd.dma_start(out=ma[3:4, :], in_=gx[mt][p0:p0 + 64, 64:128])
            nc.gpsimd.dma_start(out=ma[4:5, :],
                                in_=cgrep[blk * BLK:(blk + 1) * BLK])
            ma_t[blk] = ma

        def st_a(c):
            blk, j = c // 8, c % 8
            if c == 0:
                emit_g(0)
                emit_ma(0)
            if j == 0 and blk + 1 < NBLK:
                if (blk + 1) % 2 == 0:
                    emit_g((blk + 1) // 2)
                emit_ma(blk + 1)
            c0 = j * CHUNK
            # v at psum partitions 64:128 (fp16 col-shift), K=128
            ps_a = psp.tile([128, 512], f32, tag="ps", name=f"pa{c}")
            nc.tensor.matmul(ps_a[64:128, :], pa_t[:],
                             ma_t[blk][:, c0:c0 + CHUNK],
                             start=True, stop=True, tile_position=(0, 64))
            # t1 = (1-a)*relu(v), written back into ma2 rows 64:128
            nc.scalar.activation(ma_t[blk][64:128, c0:c0 + CHUNK],
                                 ps_a[64:128, :], AF.Relu,
                                 bias=0.0, scale=sc_t[64:128, 0:1])
            pss[c] = ps_a

        def st_b(c):
            blk, j = c // 8, c % 8
            c0 = j * CHUNK
            del pss[c]
            ps_v = psp.tile([128, 512], f32, tag="ps", name=f"pv{c}")
            nc.tensor.matmul(ps_v[:, :], pb_t[:],
                             ma_t[blk][:, c0:c0 + CHUNK],
                             start=True, stop=True)
            rr = rrp.tile([128, CHUNK], h16, tag="rr", name=f"rr{c}")
            nc.vector.tensor_scalar(
                out=rr[:, :], in0=ps_v[:, :],
                scalar1=bk_t[:, 0:1], scalar2=kk_t[:, 0:1],
                op0=ALU.add, op1=ALU.max)
            rrs[c] = rr

        def st_c(c):
            q = c % 4
            if q == 0:
                po_ps[c // 4] = pop.tile([128, 512], f32, tag="po",
                                         name=f"po{c // 4}")
            nc.tensor.matmul(po_ps[c // 4][32 * q:32 * q + 32, :], pc_t[:, :],
                             rrs[c][:, :], start=True, stop=True,
                             tile_position=(0, 32 * q))
            del rrs[c]
            if q == 3:
                po_sb = posbp.tile([97, 512], f32, tag="po_sb",
                                   name=f"po_sb{c // 4}")
                nc.scalar.activation(po_sb[0:97, :], po_ps[c // 4][0:97, :],
                                     AF.Identity, bias=0.0, scale=1.0)
                o0 = (c - 3) * CHUNK
                nc.sync.dma_start(out=out_d[o0:o0 + 4 * CHUNK],
                                  in_=po_sb[0:97:32, :])
                del po_ps[c // 4]

        for p in range(NCHUNK // 2 + 4):
            c = 2 * p
            if c < NCHUNK:
                st_a(c)
                st_a(c + 1)
            if 4 <= c < NCHUNK + 4:
                st_b(c - 4)
                st_b(c - 3)
            if c >= 8:
                st_c(c - 8)
                st_c(c - 7)

    nc.compile()
    return nc



def _get_program():
    if "prog" not in _CACHE:
        _CACHE["prog"] = _build_program()
    return _CACHE["prog"]


def kernel(x, mask, W_fs, b_fs, W_in, b_in, adj, W_gc, b_gc, W_lo, b_lo,
           prelu_a, W_ro, b_ro, W_o1, b_o1, W_o2, b_o2):
    x = np.asarray(x, np.float32)
    mask_f = np.asarray(mask, np.float16)
    adj = np.asarray(adj, np.float32)

    folded = _fold_weights(np.asarray(W_fs), np.asarray(b_fs),
                           np.asarray(W_in), np.asarray(b_in),
                           np.asarray(W_gc), np.asarray(b_gc),
                           np.asarray(W_lo), np.asarray(b_lo),
                           float(prelu_a),
                           np.asarray(W_ro), np.asarray(b_ro),
                           np.asarray(W_o1), np.asarray(b_o1),
                           np.asarray(W_o2), np.asarray(b_o2), adj)

    nc = _get_program()

    shared = dict(adj=adj.astype(np.float16), cgrep=folded["cgrep"],
                  ones_row=folded["ones_row"], zr=folded["zr"],
                  pa=folded["pa"], pb=folded["pb"], pc=folded["pc"],
                  bk=folded["bk"], kk=folded["kk"], bfs=folded["bfs"],
                  sc=folded["sc"])
    in_maps = []
    for b in range(B):
        m = dict(shared)
        m["xb"] = np.ascontiguousarray(x[b, 0]).astype(np.float16)
        m["mb"] = np.ascontiguousarray(mask_f[b, 0])
        in_maps.append(m)

    from concourse.bass_utils import run_bass_kernel_spmd
    res = run_bass_kernel_spmd(nc, in_maps, list(range(B)))

    out = np.empty((B, C, N, T), np.float32)
    for b in range(B):
        out[b, 0] = np.asarray(res.results[b]["out"]).reshape(N, T)
    return out

